# revision 7
# baseline (speedup 1.0000x reference)
"""Fused LoRA-MLP (SwiGLU) expert kernel for TRN2, 8-core expert-parallel.

Problem (per full batch): x:(8192,2048) shared-weight expert MLP
    gu  = x @ W_gu.T + 0.25 * (x @ A_gu.T) @ B_gu.T        (.,8192)
    h   = gu[:, 4096:] * silu(gu[:, :4096])                 (.,4096)
    out = h @ W_d.T  + 0.25 * (h @ A_d.T)  @ B_d.T          (.,2048)

Key trick: the LoRA factors are merged into the dense weights on the host
(W1 = W_gu + s*B_gu@A_gu, W2 = W_d + s*B_d@A_d — mathematically exact),
so the device kernel is just two dense GEMMs + SwiGLU.

Sharding: expert/data parallel — core c owns tokens [1024c, 1024(c+1)),
weights replicated per core. No collectives.

All operands are bf16 (halves HBM traffic; PE rate identical to fp32r at
full rate); PSUM accumulation is fp32. Activations flow feature-major
(xT -> hT -> outT) so no on-chip transposes are needed. mm2 accumulates
its full 32-tile contraction in PSUM (no SBUF accumulator adds).
"""

import os
from contextlib import ExitStack

import numpy as np
import ml_dtypes

import concourse.bass as bass
import concourse.bacc as bacc
import concourse.tile as tile
import concourse.mybir as mybir
from concourse.bass_utils import run_bass_kernel_spmd

F32 = mybir.dt.float32
BF16 = mybir.dt.bfloat16
NPBF16 = ml_dtypes.bfloat16
AF = mybir.ActivationFunctionType

NCORES = 8
T = 1024          # tokens per core
H = 2048          # hidden
D = 4096          # expert dim
F = 2 * D         # gate+up features
R = 64            # lora rank
SCALING = 16 / 64

KT = H // 128     # 16 k-tiles (mm1 contraction)
FT = F // 128     # 64 f-tiles (mm1 outputs)
DT = D // 128     # 32 d-tiles (mm2 contraction)
JT = H // 128     # 16 j-tiles (mm2 outputs)
TC = 512          # moving-dim chunk (PSUM bank = 512 fp32)
NCH = T // TC     # 2 chunks

_CACHE = {}


def _build(reps=1, sim_safe=False):
    # sim_safe: CoreSim lacks Silu — use Sigmoid+Copy+2 muls (same math)
    nc = bacc.Bacc("TRN2", target_bir_lowering=False, debug=False,
                   num_devices=NCORES)

    xT = nc.dram_tensor("xT", [128, KT * T], BF16, kind="ExternalInput")
    w1 = nc.dram_tensor("w1", [FT, 128, KT * 128], BF16, kind="ExternalInput")
    w2 = nc.dram_tensor("w2", [JT, 128, DT * 128], BF16, kind="ExternalInput")
    outT = nc.dram_tensor("outT", [JT, 128, T], F32, kind="ExternalOutput")

    with tile.TileContext(nc) as tc, ExitStack() as ctx:
        xpool = ctx.enter_context(tc.tile_pool(name="xpool", bufs=1))
        w1_pool = ctx.enter_context(tc.tile_pool(name="w1p", bufs=6))
        w2_pool = ctx.enter_context(tc.tile_pool(name="w2p", bufs=3))
        ht_pool = ctx.enter_context(tc.tile_pool(name="htp", bufs=DT))
        sil_pool = ctx.enter_context(tc.tile_pool(name="silp", bufs=6))
        ob_pool = ctx.enter_context(tc.tile_pool(name="obp", bufs=3))
        ps_a = ctx.enter_context(tc.tile_pool(name="psa", bufs=4, space="PSUM"))
        ps_b = ctx.enter_context(tc.tile_pool(name="psb", bufs=4, space="PSUM"))

        for rep in range(reps):
            xbuf = xpool.tile([128, KT * T], BF16)
            nc.sync.dma_start(out=xbuf[:], in_=xT[:, :])

            def xsl(k, c):
                return xbuf[:, k * T + c * TC: k * T + (c + 1) * TC]

            # ---- mm1: per f-pair (gate i, up i+DT) -> h tile (bf16)
            ht_tiles = []
            for i in range(DT):
                sg = w1_pool.tile([128, KT * 128], BF16, tag="w1")
                nc.sync.dma_start(out=sg[:], in_=w1[i])
                su = w1_pool.tile([128, KT * 128], BF16, tag="w1")
                nc.sync.dma_start(out=su[:], in_=w1[i + DT])
                ht_i = ht_pool.tile([128, T], BF16, tag="ht",
                                    name=f"ht_{rep}_{i}")
                ht_tiles.append(ht_i)
                for c in range(NCH):
                    pg = ps_a.tile([128, TC], F32, tag="psa")
                    pu = ps_a.tile([128, TC], F32, tag="psa")
                    for ps, s in ((pg, sg), (pu, su)):
                        for k in range(KT):
                            nc.tensor.matmul(
                                ps[:], s[:, k * 128:(k + 1) * 128], xsl(k, c),
                                start=(k == 0), stop=(k == KT - 1))
                    hsl = ht_i[:, c * TC:(c + 1) * TC]
                    sil = sil_pool.tile([128, TC], F32, tag="sil")
                    if sim_safe:
                        nc.scalar.activation(sil[:], pg[:], AF.Sigmoid)
                        gcp = sil_pool.tile([128, TC], F32, tag="sil")
                        nc.scalar.activation(gcp[:], pg[:], AF.Copy)
                        ug = sil_pool.tile([128, TC], F32, tag="sil")
                        nc.vector.tensor_mul(ug[:], pu[:], sil[:])
                        nc.vector.tensor_mul(hsl, ug[:], gcp[:])
                    else:
                        nc.scalar.activation(sil[:], pg[:], AF.Silu)
                        nc.vector.tensor_mul(hsl, pu[:], sil[:])

            # ---- mm2: full 32-deep contraction accumulated in PSUM
            for j in range(JT):
                s2 = w2_pool.tile([128, DT * 128], BF16, tag="w2")
                nc.sync.dma_start(out=s2[:], in_=w2[j])
                ob = ob_pool.tile([128, T], F32, tag="ob")
                for c in range(NCH):
                    ps = ps_b.tile([128, TC], F32, tag="psb")
                    for d in range(DT):
                        nc.tensor.matmul(
                            ps[:], s2[:, d * 128:(d + 1) * 128],
                            ht_tiles[d][:, c * TC:(c + 1) * TC],
                            start=(d == 0), stop=(d == DT - 1))
                    nc.vector.tensor_copy(ob[:, c * TC:(c + 1) * TC], ps[:])
                nc.sync.dma_start(out=outT[j], in_=ob[:])

    nc.compile()
    return nc


def _prep_shared(W_gu, A_gu, B_gu, W_d, A_d, B_d):
    # merge LoRA into the dense weights (exact), then tile to the
    # stationary layout: w[m, p, k*128 + f] = W[m*128+f, k*128+p]
    W1 = W_gu + SCALING * (B_gu @ A_gu)          # (F, H)
    W2 = W_d + SCALING * (B_d @ A_d)             # (H, D)
    w1_t = W1.reshape(FT, 128, KT, 128).transpose(0, 3, 2, 1).astype(
        NPBF16).reshape(FT, 128, KT * 128)
    w2_t = W2.reshape(JT, 128, DT, 128).transpose(0, 3, 2, 1).astype(
        NPBF16).reshape(JT, 128, DT * 128)
    return dict(w1=w1_t, w2=w2_t)


def _prep_x(hidden_states):
    # per-core xT pre-tiled as [p, k, t] flattened to [128, KT*T]
    return hidden_states.reshape(NCORES, T, KT, 128).transpose(
        0, 3, 2, 1).astype(NPBF16).reshape(NCORES, 128, KT * T)


def kernel(hidden_states, W_gu, A_gu, B_gu, W_d, A_d, B_d):
    hidden_states = np.asarray(hidden_states, dtype=np.float32)
    shared = _prep_shared(*(np.asarray(a, dtype=np.float32)
                            for a in (W_gu, A_gu, B_gu, W_d, A_d, B_d)))
    xt = _prep_x(hidden_states)

    if "nc" not in _CACHE:
        _CACHE["nc"] = _build()
    nc = _CACHE["nc"]

    in_maps = [dict(shared, xT=xt[c]) for c in range(NCORES)]
    trace = os.environ.get("KERNEL_TRACE", "0") == "1"
    res = run_bass_kernel_spmd(nc, in_maps, list(range(NCORES)), trace=trace)
    _CACHE["last_result"] = res

    out = np.empty((NCORES, T, H), np.float32)
    for c in range(NCORES):
        o = res.results[c]["outT"].reshape(JT, 128, T)
        out[c] = o.transpose(2, 0, 1).reshape(T, H)
    return out.reshape(NCORES * T, H)


# revision 9
# speedup vs baseline: 5.4457x; 5.4457x over previous
"""Fused LoRA-MLP (SwiGLU) expert kernel for TRN2, 8-core expert-parallel.

Problem (per full batch): x:(8192,2048) shared-weight expert MLP
    gu  = x @ W_gu.T + 0.25 * (x @ A_gu.T) @ B_gu.T        (.,8192)
    h   = gu[:, 4096:] * silu(gu[:, :4096])                 (.,4096)
    out = h @ W_d.T  + 0.25 * (h @ A_d.T)  @ B_d.T          (.,2048)

Key trick: the LoRA factors are merged into the dense weights on the host
(W1 = W_gu + s*B_gu@A_gu, W2 = W_d + s*B_d@A_d — mathematically exact),
so the device kernel is just two dense GEMMs + SwiGLU.

Sharding: expert/data parallel — core c owns tokens [1024c, 1024(c+1)),
weights replicated per core. No collectives.

All operands are bf16 (halves HBM traffic; PE rate identical to fp32r at
full rate); PSUM accumulation is fp32. Activations flow feature-major
(xT -> hT -> outT) so no on-chip transposes are needed. mm2 accumulates
its full 32-tile contraction in PSUM (no SBUF accumulator adds).
"""

import os
from contextlib import ExitStack

import numpy as np
import ml_dtypes

import concourse.bass as bass
import concourse.bacc as bacc
import concourse.tile as tile
import concourse.mybir as mybir
from concourse.bass_utils import run_bass_kernel_spmd

F32 = mybir.dt.float32
BF16 = mybir.dt.bfloat16
NPBF16 = ml_dtypes.bfloat16
AF = mybir.ActivationFunctionType

NCORES = 8
T = 1024          # tokens per core
H = 2048          # hidden
D = 4096          # expert dim
F = 2 * D         # gate+up features
R = 64            # lora rank
SCALING = 16 / 64

KT = H // 128     # 16 k-tiles (mm1 contraction)
FT = F // 128     # 64 f-tiles (mm1 outputs)
DT = D // 128     # 32 d-tiles (mm2 contraction)
JT = H // 128     # 16 j-tiles (mm2 outputs)
TC = 512          # moving-dim chunk (PSUM bank = 512 fp32)
NCH = T // TC     # 2 chunks

_CACHE = {}


def _build(reps=1, sim_safe=False):
    # sim_safe: CoreSim lacks Silu — use Sigmoid+Copy+2 muls (same math)
    nc = bacc.Bacc("TRN2", target_bir_lowering=False, debug=False,
                   num_devices=NCORES)

    xT = nc.dram_tensor("xT", [128, KT * T], BF16, kind="ExternalInput")
    w1 = nc.dram_tensor("w1", [FT, 128, KT * 128], BF16, kind="ExternalInput")
    w2 = nc.dram_tensor("w2", [JT, 128, DT * 128], BF16, kind="ExternalInput")
    outT = nc.dram_tensor("outT", [JT, 128, T], F32, kind="ExternalOutput")

    with tile.TileContext(nc) as tc, ExitStack() as ctx:
        xpool = ctx.enter_context(tc.tile_pool(name="xpool", bufs=KT))
        w1_pool = ctx.enter_context(tc.tile_pool(name="w1p", bufs=8))
        w2_pool = ctx.enter_context(tc.tile_pool(name="w2p", bufs=3))
        ht_pool = ctx.enter_context(tc.tile_pool(name="htp", bufs=DT))
        sil_pool = ctx.enter_context(tc.tile_pool(name="silp", bufs=6))
        ob_pool = ctx.enter_context(tc.tile_pool(name="obp", bufs=3))
        ps_a = ctx.enter_context(tc.tile_pool(name="psa", bufs=4, space="PSUM"))
        ps_b = ctx.enter_context(tc.tile_pool(name="psb", bufs=4, space="PSUM"))

        for rep in range(reps):
            # per-k-tile x loads so mm1 starts as soon as k=0 lands
            xk = []
            for k in range(KT):
                xt_k = xpool.tile([128, T], BF16, tag="x",
                                  name=f"x_{rep}_{k}")
                nc.sync.dma_start(out=xt_k[:], in_=xT[:, k * T:(k + 1) * T])
                xk.append(xt_k)

            def xsl(k, c):
                return xk[k][:, c * TC:(c + 1) * TC]

            # ---- mm1: per f-pair (gate i, up i+DT) -> h tile (bf16)
            ht_tiles = []
            for i in range(DT):
                sg = w1_pool.tile([128, KT * 128], BF16, tag="w1")
                nc.sync.dma_start(out=sg[:], in_=w1[i])
                su = w1_pool.tile([128, KT * 128], BF16, tag="w1")
                nc.sync.dma_start(out=su[:], in_=w1[i + DT])
                ht_i = ht_pool.tile([128, T], BF16, tag="ht",
                                    name=f"ht_{rep}_{i}")
                ht_tiles.append(ht_i)
                for c in range(NCH):
                    pg = ps_a.tile([128, TC], F32, tag="psa")
                    pu = ps_a.tile([128, TC], F32, tag="psa")
                    for ps, s in ((pg, sg), (pu, su)):
                        for k in range(KT):
                            nc.tensor.matmul(
                                ps[:], s[:, k * 128:(k + 1) * 128], xsl(k, c),
                                start=(k == 0), stop=(k == KT - 1))
                    hsl = ht_i[:, c * TC:(c + 1) * TC]
                    sil = sil_pool.tile([128, TC], F32, tag="sil")
                    if sim_safe:
                        nc.scalar.activation(sil[:], pg[:], AF.Sigmoid)
                        gcp = sil_pool.tile([128, TC], F32, tag="sil")
                        nc.scalar.activation(gcp[:], pg[:], AF.Copy)
                        ug = sil_pool.tile([128, TC], F32, tag="sil")
                        nc.vector.tensor_mul(ug[:], pu[:], sil[:])
                        nc.vector.tensor_mul(hsl, ug[:], gcp[:])
                    else:
                        nc.scalar.activation(sil[:], pg[:], AF.Silu)
                        nc.vector.tensor_mul(hsl, pu[:], sil[:])

            # ---- mm2: full 32-deep contraction accumulated in PSUM
            for j in range(JT):
                s2 = w2_pool.tile([128, DT * 128], BF16, tag="w2")
                nc.sync.dma_start(out=s2[:], in_=w2[j])
                ob = ob_pool.tile([128, T], F32, tag="ob")
                for c in range(NCH):
                    ps = ps_b.tile([128, TC], F32, tag="psb")
                    for d in range(DT):
                        nc.tensor.matmul(
                            ps[:], s2[:, d * 128:(d + 1) * 128],
                            ht_tiles[d][:, c * TC:(c + 1) * TC],
                            start=(d == 0), stop=(d == DT - 1))
                    nc.vector.tensor_copy(ob[:, c * TC:(c + 1) * TC], ps[:])
                nc.sync.dma_start(out=outT[j], in_=ob[:])

    nc.compile()
    return nc


def _prep_shared(W_gu, A_gu, B_gu, W_d, A_d, B_d):
    # merge LoRA into the dense weights (exact), then tile to the
    # stationary layout: w[m, p, k*128 + f] = W[m*128+f, k*128+p]
    W1 = W_gu + SCALING * (B_gu @ A_gu)          # (F, H)
    W2 = W_d + SCALING * (B_d @ A_d)             # (H, D)
    w1_t = W1.reshape(FT, 128, KT, 128).transpose(0, 3, 2, 1).astype(
        NPBF16).reshape(FT, 128, KT * 128)
    w2_t = W2.reshape(JT, 128, DT, 128).transpose(0, 3, 2, 1).astype(
        NPBF16).reshape(JT, 128, DT * 128)
    return dict(w1=w1_t, w2=w2_t)


def _prep_x(hidden_states):
    # per-core xT pre-tiled as [p, k, t] flattened to [128, KT*T]
    return hidden_states.reshape(NCORES, T, KT, 128).transpose(
        0, 3, 2, 1).astype(NPBF16).reshape(NCORES, 128, KT * T)


def kernel(hidden_states, W_gu, A_gu, B_gu, W_d, A_d, B_d):
    hidden_states = np.asarray(hidden_states, dtype=np.float32)
    shared = _prep_shared(*(np.asarray(a, dtype=np.float32)
                            for a in (W_gu, A_gu, B_gu, W_d, A_d, B_d)))
    xt = _prep_x(hidden_states)

    if "nc" not in _CACHE:
        _CACHE["nc"] = _build()
    nc = _CACHE["nc"]

    in_maps = [dict(shared, xT=xt[c]) for c in range(NCORES)]
    trace = os.environ.get("KERNEL_TRACE", "0") == "1"
    res = run_bass_kernel_spmd(nc, in_maps, list(range(NCORES)), trace=trace)
    _CACHE["last_result"] = res

    out = np.empty((NCORES, T, H), np.float32)
    for c in range(NCORES):
        o = res.results[c]["outT"].reshape(JT, 128, T)
        out[c] = o.transpose(2, 0, 1).reshape(T, H)
    return out.reshape(NCORES * T, H)
